# revision 7
# baseline (speedup 1.0000x reference)
"""Trainium2 Bass kernel for C = triu(A @ B), A/B upper-triangular 4096x4096 f32.

Contract: kernel(**inputs) takes FULL inputs {"A": [4096,4096] f32, "B": ...},
returns FULL [4096,4096] f32 output. Internally shards across 8 NeuronCores
via run_bass_kernel_spmd (SPMD: one program, per-core data).

v0: dense row-sharded matmul (no triangular skipping yet).
Each core c computes C[512c:512c+512, :] = A[512c:512c+512, :] @ B.
Since A and B are upper triangular, A @ B is exactly upper triangular
(every sub-diagonal term has a 0.0 factor), so no output masking is needed.
"""

import sys

sys.path.insert(0, "/opt/trn_rl_repo")

import numpy as np

N = 4096
N_CORES = 8
ROWS_PER_CORE = N // N_CORES  # 512
MB = ROWS_PER_CORE // 128  # 4 local row-blocks of 128
KB = N // 128  # 32 k-blocks
JG = N // 512  # 8 column groups of 512

_cache = {}


def _build_nc():
    import concourse.bacc as bacc
    import concourse.mybir as mybir
    import concourse.tile as tile

    nc = bacc.Bacc()
    at = nc.declare_dram_parameter("AT", [N, ROWS_PER_CORE], mybir.dt.float32,
                                   isOutput=False)
    b = nc.declare_dram_parameter("B", [N, N], mybir.dt.float32, isOutput=False)
    c = nc.declare_dram_parameter("C", [ROWS_PER_CORE, N], mybir.dt.float32,
                                  isOutput=True)

    # AT[k, m] viewed as [p, kb, m] with k = kb*128 + p
    at_v = at.rearrange("(kb p) m -> p kb m", p=128)

    with tile.TileContext(nc) as tc:
        with (
            tc.tile_pool(name="a", bufs=1) as a_pool,
            tc.tile_pool(name="bt", bufs=4) as b_pool,
            tc.tile_pool(name="co", bufs=8) as c_pool,
            tc.tile_pool(name="ps", bufs=2, space="PSUM") as ps_pool,
        ):
            # Load all of AT for this core into SBUF: 4 tiles [128, KB*128]
            a_tiles = []
            for l in range(MB):
                a_t = a_pool.tile([128, KB, 128], mybir.dt.float32, tag=f"a{l}")
                nc.sync.dma_start(out=a_t[:], in_=at_v[:, :, l * 128:(l + 1) * 128])
                a_tiles.append(a_t)

            for jg in range(JG):
                ps_tiles = [
                    ps_pool.tile([128, 512], mybir.dt.float32, tag=f"ps{l}",
                                 name=f"ps_{jg}_{l}")
                    for l in range(MB)
                ]
                for kb in range(KB):
                    b_t = b_pool.tile([128, 512], mybir.dt.float32, tag="b")
                    nc.sync.dma_start(
                        out=b_t[:],
                        in_=b[kb * 128:(kb + 1) * 128, jg * 512:(jg + 1) * 512],
                    )
                    for l in range(MB):
                        nc.tensor.matmul(
                            ps_tiles[l][:],
                            lhsT=a_tiles[l][:, kb, :],
                            rhs=b_t[:],
                            start=(kb == 0),
                            stop=(kb == KB - 1),
                        )
                for l in range(MB):
                    c_t = c_pool.tile([128, 512], mybir.dt.float32, tag="c",
                                      name=f"c_{jg}_{l}")
                    nc.vector.tensor_copy(c_t[:], ps_tiles[l][:])
                    nc.sync.dma_start(
                        out=c[l * 128:(l + 1) * 128, jg * 512:(jg + 1) * 512],
                        in_=c_t[:],
                    )
    nc.finalize()
    return nc


def kernel(A: np.ndarray, B: np.ndarray) -> np.ndarray:
    from concourse.bass_utils import run_bass_kernel_spmd

    A = np.asarray(A, dtype=np.float32)
    B = np.asarray(B, dtype=np.float32)

    if "nc" not in _cache:
        _cache["nc"] = _build_nc()
    nc = _cache["nc"]

    in_maps = []
    for c in range(N_CORES):
        rows = slice(c * ROWS_PER_CORE, (c + 1) * ROWS_PER_CORE)
        at = np.ascontiguousarray(A[rows, :].T)
        in_maps.append({"AT": at, "B": B})

    res = run_bass_kernel_spmd(nc, in_maps, list(range(N_CORES)))
    out = np.concatenate([res.results[c]["C"] for c in range(N_CORES)], axis=0)
    return out.astype(np.float32, copy=False)


def _make_in_maps(A, B):
    in_maps = []
    for c in range(N_CORES):
        rows = slice(c * ROWS_PER_CORE, (c + 1) * ROWS_PER_CORE)
        at = np.ascontiguousarray(A[rows, :].T)
        in_maps.append({"AT": at, "B": B})
    return in_maps


def hw_exec_time_ns(np_inputs, reps=5):
    """Test-only: time the compiled NEFF via PJRT with device-resident
    inputs (min over reps). Mirrors bass2jax.run_bass_via_pjrt."""
    import time
    import jax
    import jax.numpy as jnp
    from jax.sharding import Mesh, PartitionSpec, NamedSharding
    from jax.experimental.shard_map import shard_map
    from concourse import bass2jax, mybir

    A = np.asarray(np_inputs["A"], dtype=np.float32)
    B = np.asarray(np_inputs["B"], dtype=np.float32)
    if "nc" not in _cache:
        _cache["nc"] = _build_nc()
    nc = _cache["nc"]
    bass2jax.install_neuronx_cc_hook()

    partition_name = nc.partition_id_tensor.name if nc.partition_id_tensor else None
    in_names, out_names, out_avals, zero_outs = [], [], [], []
    for alloc in nc.m.functions[0].allocations:
        if not isinstance(alloc, mybir.MemoryLocationSet):
            continue
        name = alloc.memorylocations[0].name
        if alloc.kind == "ExternalInput":
            if name != partition_name:
                in_names.append(name)
        elif alloc.kind == "ExternalOutput":
            shape = tuple(alloc.tensor_shape)
            dtype = mybir.dt.np(alloc.dtype)
            out_avals.append(jax.core.ShapedArray(shape, dtype))
            zero_outs.append(np.zeros(shape, dtype))
    n_params = len(in_names)
    n_outs = len(out_avals)
    all_in_names = list(in_names) + out_names
    for alloc in nc.m.functions[0].allocations:
        if isinstance(alloc, mybir.MemoryLocationSet) and alloc.kind == "ExternalOutput":
            all_in_names.append(alloc.memorylocations[0].name)
            out_names.append(alloc.memorylocations[0].name)
    if partition_name is not None:
        all_in_names.append(partition_name)

    def _body(*args):
        operands = list(args)
        if partition_name is not None:
            operands.append(bass2jax.partition_id_tensor())
        outs = bass2jax._bass_exec_p.bind(
            *operands,
            out_avals=tuple(out_avals),
            in_names=tuple(all_in_names),
            out_names=tuple(out_names),
            lowering_input_output_aliases=(),
            sim_require_finite=True,
            sim_require_nnan=True,
            nc=nc,
        )
        return tuple(outs)

    devices = jax.devices()[:N_CORES]
    mesh = Mesh(np.asarray(devices), ("core",))
    in_specs = (PartitionSpec("core"),) * (n_params + n_outs)
    out_specs = (PartitionSpec("core"),) * n_outs
    sharded = jax.jit(
        shard_map(_body, mesh=mesh, in_specs=in_specs, out_specs=out_specs,
                  check_rep=False),
        donate_argnums=tuple(range(n_params, n_params + n_outs)),
        keep_unused=True,
    )

    in_maps = _make_in_maps(A, B)
    per_core = [[np.asarray(m[name]) for name in in_names] for m in in_maps]
    concat_in = [
        np.concatenate([per_core[c][i] for c in range(N_CORES)], axis=0)
        for i in range(n_params)
    ]
    shard = NamedSharding(mesh, PartitionSpec("core"))
    staged = [jax.device_put(x, shard) for x in concat_in]
    for x in staged:
        x.block_until_ready()

    times = []
    out = None
    for _ in range(reps):
        zeros = [
            jax.device_put(np.zeros((N_CORES * z.shape[0], *z.shape[1:]), z.dtype),
                           shard)
            for z in zero_outs
        ]
        for z in zeros:
            z.block_until_ready()
        t0 = time.perf_counter()
        out = sharded(*staged, *zeros)
        jax.block_until_ready(out)
        t1 = time.perf_counter()
        times.append(t1 - t0)
    print("exec times (ms):", [f"{t*1e3:.3f}" for t in times])
    return int(min(times) * 1e9)


# revision 8
# speedup vs baseline: 85.4221x; 85.4221x over previous
"""Trainium2 Bass kernel for C = triu(A @ B), A/B upper-triangular 4096x4096 f32.

Contract: kernel(**inputs) takes FULL inputs {"A": [4096,4096] f32, "B": ...},
returns FULL [4096,4096] f32 output. Internally shards across 8 NeuronCores
via run_bass_kernel_spmd (SPMD: one program, per-core data).

v0: dense row-sharded matmul (no triangular skipping yet).
Each core c computes C[512c:512c+512, :] = A[512c:512c+512, :] @ B.
Since A and B are upper triangular, A @ B is exactly upper triangular
(every sub-diagonal term has a 0.0 factor), so no output masking is needed.
"""

import sys

sys.path.insert(0, "/opt/trn_rl_repo")

import numpy as np

N = 4096
N_CORES = 8
ROWS_PER_CORE = N // N_CORES  # 512
MB = ROWS_PER_CORE // 128  # 4 local row-blocks of 128
KB = N // 128  # 32 k-blocks
JG = N // 512  # 8 column groups of 512

_cache = {}


def _build_nc():
    import concourse.bacc as bacc
    import concourse.mybir as mybir
    import concourse.tile as tile

    nc = bacc.Bacc()
    at = nc.declare_dram_parameter("AT", [N, ROWS_PER_CORE], mybir.dt.float32,
                                   isOutput=False)
    b = nc.declare_dram_parameter("B", [N, N], mybir.dt.float32, isOutput=False)
    c = nc.declare_dram_parameter("C", [ROWS_PER_CORE, N], mybir.dt.float32,
                                  isOutput=True)

    # AT[k, m] viewed as [p, kb, m] with k = kb*128 + p
    at_v = at.rearrange("(kb p) m -> p kb m", p=128)

    with tile.TileContext(nc) as tc:
        with (
            tc.tile_pool(name="a", bufs=1) as a_pool,
            tc.tile_pool(name="bt", bufs=4) as b_pool,
            tc.tile_pool(name="co", bufs=8) as c_pool,
            tc.tile_pool(name="ps", bufs=2, space="PSUM") as ps_pool,
        ):
            # Load all of AT for this core into SBUF: 4 tiles [128, KB*128]
            a_tiles = []
            for l in range(MB):
                a_t = a_pool.tile([128, KB, 128], mybir.dt.float32, tag=f"a{l}")
                nc.sync.dma_start(out=a_t[:], in_=at_v[:, :, l * 128:(l + 1) * 128])
                a_tiles.append(a_t)

            for jg in range(JG):
                ps_tiles = [
                    ps_pool.tile([128, 512], mybir.dt.float32, tag=f"ps{l}",
                                 name=f"ps_{jg}_{l}")
                    for l in range(MB)
                ]
                for kb in range(KB):
                    b_t = b_pool.tile([128, 512], mybir.dt.float32, tag="b")
                    nc.sync.dma_start(
                        out=b_t[:],
                        in_=b[kb * 128:(kb + 1) * 128, jg * 512:(jg + 1) * 512],
                    )
                    for l in range(MB):
                        nc.tensor.matmul(
                            ps_tiles[l][:],
                            lhsT=a_tiles[l][:, kb, :],
                            rhs=b_t[:],
                            start=(kb == 0),
                            stop=(kb == KB - 1),
                        )
                for l in range(MB):
                    c_t = c_pool.tile([128, 512], mybir.dt.float32, tag="c",
                                      name=f"c_{jg}_{l}")
                    nc.vector.tensor_copy(c_t[:], ps_tiles[l][:])
                    nc.sync.dma_start(
                        out=c[l * 128:(l + 1) * 128, jg * 512:(jg + 1) * 512],
                        in_=c_t[:],
                    )
    nc.finalize()
    return nc


def kernel(A: np.ndarray, B: np.ndarray) -> np.ndarray:
    from concourse.bass_utils import run_bass_kernel_spmd

    A = np.asarray(A, dtype=np.float32)
    B = np.asarray(B, dtype=np.float32)

    if "nc" not in _cache:
        _cache["nc"] = _build_nc()
    nc = _cache["nc"]

    in_maps = []
    for c in range(N_CORES):
        rows = slice(c * ROWS_PER_CORE, (c + 1) * ROWS_PER_CORE)
        at = np.ascontiguousarray(A[rows, :].T)
        in_maps.append({"AT": at, "B": B})

    res = run_bass_kernel_spmd(nc, in_maps, list(range(N_CORES)))
    out = np.concatenate([res.results[c]["C"] for c in range(N_CORES)], axis=0)
    return out.astype(np.float32, copy=False)


def _make_in_maps(A, B):
    in_maps = []
    for c in range(N_CORES):
        rows = slice(c * ROWS_PER_CORE, (c + 1) * ROWS_PER_CORE)
        at = np.ascontiguousarray(A[rows, :].T)
        in_maps.append({"AT": at, "B": B})
    return in_maps


def get_nc():
    if "nc" not in _cache:
        _cache["nc"] = _build_nc()
    return _cache["nc"]


# revision 9
# speedup vs baseline: 359.1485x; 4.2044x over previous
"""Trainium2 Bass kernel for C = triu(A @ B), A/B upper-triangular 4096x4096 f32.

kernel(**inputs) takes FULL inputs {"A","B"} and returns the FULL output,
sharding across 8 NeuronCores via run_bass_kernel_spmd (SPMD: one program,
per-core data).

Design (v1, triangular-skipping):
  C is tiled into 128x512 "supers" (bi = 128-row block 0..31, jg = 512-col
  group 0..7). Super (bi,jg) is nonzero only when jg >= bi//4, and needs
  contraction over k-blocks bk in [bi, 4jg+3]: depth d = 4jg+4-bi. Since A
  and B are upper triangular, A@B is exactly upper triangular, so skipped
  supers are exact zeros (outputs are pre-zeroed by the runner).

  Work unit = "pair sweep": two supers of the SAME column jg accumulate in
  two PSUM banks over one shared B stream (bk ascending, right-aligned).
  The diagonal steps (last 3 of a column) use narrow matmuls (384/256/128
  cols) matching B's nonzero columns exactly.

  SPMD uniformity: all 8 cores run one static template: 9 slots, slot s =
  (L_s steps, psum2 engages at step off_s). A core's real pair (depths
  l1 >= l2) is right-aligned into the slot; leading pad matmuls use
  zero stationary blocks (packed inline) and contribute exact 0.0.
  All cores execute identical instruction streams on different packed data
  => perfect load balance.

  Per-core packed inputs (host-built):
    S:   [steps, 128, 768] f32  -- per step: cols 0:128 = A1^T block,
         128:256 = A2^T block (or zeros), 256:768 = B block [128,512].
         One fused DMA per step (3KB per partition line).
    Out: CP [2*nslots, 128, 512] f32, host scatters valid columns into C.
"""

import sys

sys.path.insert(0, "/opt/trn_rl_repo")

import numpy as np

N = 4096
N_CORES = 8
NB = N // 128          # 32 row blocks
NJ = N // 512          # 8 column groups


def _build_schedule():
    """Static schedule, identical for all cores.

    Returns:
      slots: list of (L, off) sweep templates.
      assign: assign[core][s] = (jg, bi1, bi2) the real pair for that slot.
    """
    # pairs within each column: adjacent rows (bi=2t, 2t+1); depths
    # (d1, d2) = (4jg+4-2t, 4jg+3-2t)
    pairs = []  # (l1, l2, jg, bi1, bi2)
    for jg in range(NJ):
        rows = 4 * jg + 4
        for t in range(rows // 2):
            bi1, bi2 = 2 * t, 2 * t + 1
            pairs.append((4 * jg + 4 - bi1, 4 * jg + 4 - bi2, jg, bi1, bi2))
    assert len(pairs) == 72
    # sort by l1 desc, group into slots of 8 (one pair per core per slot)
    pairs.sort(key=lambda p: (-p[0], -p[1]))
    slots = []
    assign = [[] for _ in range(N_CORES)]
    for s in range(len(pairs) // N_CORES):
        grp = pairs[s * N_CORES:(s + 1) * N_CORES]
        L = max(p[0] for p in grp)
        l2max = max(p[1] for p in grp)
        off = L - l2max
        slots.append((L, off))
        for c in range(N_CORES):
            l1, l2, jg, bi1, bi2 = grp[c]
            assign[c].append((jg, bi1, bi2, l1, l2))
    return slots, assign


_SLOTS, _ASSIGN = _build_schedule()
_TOTAL_STEPS = sum(L for L, _ in _SLOTS)


def _step_width(t, L):
    """Moving-operand width (cols) at step t of an L-step sweep.

    Right-aligned: the final 3 steps of a column hit the diagonal B blocks
    (bk = 4jg+1..3) whose nonzero columns are the right 384/256/128.
    """
    rem = L - 1 - t  # steps remaining after this one
    if rem == 0:
        return 128
    if rem == 1:
        return 256
    if rem == 2:
        return 384
    return 512


_cache = {}


def _build_nc():
    import concourse.bacc as bacc
    import concourse.mybir as mybir
    import concourse.tile as tile

    f32 = mybir.dt.float32
    nc = bacc.Bacc()
    s_in = nc.declare_dram_parameter("S", [_TOTAL_STEPS, 128, 768], f32,
                                     isOutput=False)
    cp = nc.declare_dram_parameter("CP", [2 * len(_SLOTS), 128, 512], f32,
                                   isOutput=True)

    with tile.TileContext(nc) as tc:
        with (
            tc.tile_pool(name="st", bufs=6) as s_pool,
            tc.tile_pool(name="co", bufs=4) as c_pool,
            tc.tile_pool(name="ps", bufs=2, space="PSUM") as ps_pool,
        ):
            cursor = 0
            for s, (L, off) in enumerate(_SLOTS):
                ps1 = ps_pool.tile([128, 512], f32, tag="p1", name=f"ps1_{s}")
                ps2 = ps_pool.tile([128, 512], f32, tag="p2", name=f"ps2_{s}")
                for t in range(L):
                    w = _step_width(t, L)
                    oc = 512 - w
                    st = s_pool.tile([128, 768], f32, tag="s", name=f"st_{s}_{t}")
                    nc.sync.dma_start(out=st[:], in_=s_in[cursor])
                    nc.tensor.matmul(
                        ps1[:, oc:], lhsT=st[:, 0:128], rhs=st[:, 256 + oc:768],
                        start=(t == 0), stop=(t == L - 1),
                    )
                    if t >= off:
                        nc.tensor.matmul(
                            ps2[:, oc:], lhsT=st[:, 128:256],
                            rhs=st[:, 256 + oc:768],
                            start=(t == off), stop=(t == L - 1),
                        )
                    cursor += 1
                c1 = c_pool.tile([128, 512], f32, tag="c", name=f"c1_{s}")
                nc.vector.tensor_copy(c1[:], ps1[:])
                nc.sync.dma_start(out=cp[2 * s], in_=c1[:])
                c2 = c_pool.tile([128, 512], f32, tag="c", name=f"c2_{s}")
                nc.vector.tensor_copy(c2[:], ps2[:])
                nc.sync.dma_start(out=cp[2 * s + 1], in_=c2[:])
            assert cursor == _TOTAL_STEPS
    nc.finalize()
    return nc


def get_nc():
    if "nc" not in _cache:
        _cache["nc"] = _build_nc()
    return _cache["nc"]


def _pack_core(c, A4, B4):
    """Build the fused per-step stream S for core c.

    A4[bi, bk] = 128x128 block of A^T (A4[bi,bk][p,m] = A[128bi+m, 128bk+p]).
    B4[bk, jg] = 128x512 block of B.
    """
    S = np.zeros((_TOTAL_STEPS, 128, 768), dtype=np.float32)
    cursor = 0
    for s, (L, off) in enumerate(_SLOTS):
        jg, bi1, bi2, l1, l2 = _ASSIGN[c][s]
        base = 4 * jg + 4 - L  # bk at template step 0
        for t in range(L):
            bk = base + t
            row = S[cursor]
            if bk >= bi1:
                row[:, 0:128] = A4[bi1, bk]
            if t >= off and bk >= bi2:
                row[:, 128:256] = A4[bi2, bk]
            if bk >= bi1:  # steps before bi1 never feed a real matmul
                row[:, 256:768] = B4[bk, jg]
            cursor += 1
    return S


def kernel(A: np.ndarray, B: np.ndarray) -> np.ndarray:
    from concourse.bass_utils import run_bass_kernel_spmd

    A = np.asarray(A, dtype=np.float32)
    B = np.asarray(B, dtype=np.float32)

    nc = get_nc()

    # Block views for packing
    A4 = A.reshape(NB, 128, NB, 128).transpose(0, 2, 3, 1)  # [bi, bk] -> A_blk^T
    B4 = B.reshape(NB, 128, NJ, 512).transpose(0, 2, 1, 3)  # [bk, jg] -> B_blk

    in_maps = [{"S": _pack_core(c, A4, B4)} for c in range(N_CORES)]

    res = run_bass_kernel_spmd(nc, in_maps, list(range(N_CORES)))

    C = np.zeros((N, N), dtype=np.float32)
    for c in range(N_CORES):
        cpk = res.results[c]["CP"]
        for s in range(len(_SLOTS)):
            jg, bi1, bi2, l1, l2 = _ASSIGN[c][s]
            for track, bi in ((0, bi1), (1, bi2)):
                blk = cpk[2 * s + track]
                # valid columns of this super: j >= 128*bi (left of the
                # diagonal is exact zero / possibly pad garbage)
                lo = max(0, 128 * bi - 512 * jg)
                C[128 * bi:128 * (bi + 1), 512 * jg + lo:512 * (jg + 1)] = \
                    blk[:, lo:]
    return C


def _make_in_maps(A, B):
    A = np.asarray(A, dtype=np.float32)
    B = np.asarray(B, dtype=np.float32)
    A4 = A.reshape(NB, 128, NB, 128).transpose(0, 2, 3, 1)
    B4 = B.reshape(NB, 128, NJ, 512).transpose(0, 2, 1, 3)
    return [{"S": _pack_core(c, A4, B4)} for c in range(N_CORES)]


# revision 13
# speedup vs baseline: 417.8752x; 1.1635x over previous
"""Trainium2 Bass kernel for C = triu(A @ B), A/B upper-triangular 4096x4096 f32.

kernel(**inputs) takes FULL inputs {"A","B"} and returns the FULL output,
sharding across 8 NeuronCores via run_bass_kernel_spmd (SPMD: one program,
per-core data).

Design (v1, triangular-skipping):
  C is tiled into 128x512 "supers" (bi = 128-row block 0..31, jg = 512-col
  group 0..7). Super (bi,jg) is nonzero only when jg >= bi//4, and needs
  contraction over k-blocks bk in [bi, 4jg+3]: depth d = 4jg+4-bi. Since A
  and B are upper triangular, A@B is exactly upper triangular, so skipped
  supers are exact zeros (outputs are pre-zeroed by the runner).

  Work unit = "pair sweep": two supers of the SAME column jg accumulate in
  two PSUM banks over one shared B stream (bk ascending, right-aligned).
  The diagonal steps (last 3 of a column) use narrow matmuls (384/256/128
  cols) matching B's nonzero columns exactly.

  SPMD uniformity: all 8 cores run one static template: 9 slots, slot s =
  (L_s steps, psum2 engages at step off_s). A core's real pair (depths
  l1 >= l2) is right-aligned into the slot; leading pad matmuls use
  zero stationary blocks (packed inline) and contribute exact 0.0.
  All cores execute identical instruction streams on different packed data
  => perfect load balance.

  Per-core packed inputs (host-built):
    S:   [steps, 128, 768] f32  -- per step: cols 0:128 = A1^T block,
         128:256 = A2^T block (or zeros), 256:768 = B block [128,512].
         One fused DMA per step (3KB per partition line).
    Out: CP [2*nslots, 128, 512] f32, host scatters valid columns into C.
"""

import sys

sys.path.insert(0, "/opt/trn_rl_repo")

import numpy as np

N = 4096
N_CORES = 8
NB = N // 128          # 32 row blocks
NJ = N // 512          # 8 column groups


def _build_schedule():
    """Static schedule, identical for all cores.

    Returns:
      slots: list of (L, off) sweep templates.
      assign: assign[core][s] = (jg, bi1, bi2) the real pair for that slot.
    """
    # pairs within each column: adjacent rows (bi=2t, 2t+1); depths
    # (d1, d2) = (4jg+4-2t, 4jg+3-2t)
    pairs = []  # (l1, l2, jg, bi1, bi2)
    for jg in range(NJ):
        rows = 4 * jg + 4
        for t in range(rows // 2):
            bi1, bi2 = 2 * t, 2 * t + 1
            pairs.append((4 * jg + 4 - bi1, 4 * jg + 4 - bi2, jg, bi1, bi2))
    assert len(pairs) == 72
    # sort by l1 desc, group into slots of 8 (one pair per core per slot)
    pairs.sort(key=lambda p: (-p[0], -p[1]))
    slots = []
    assign = [[] for _ in range(N_CORES)]
    for s in range(len(pairs) // N_CORES):
        grp = pairs[s * N_CORES:(s + 1) * N_CORES]
        L = max(p[0] for p in grp)
        l2max = max(p[1] for p in grp)
        off = L - l2max
        slots.append((L, off))
        for c in range(N_CORES):
            l1, l2, jg, bi1, bi2 = grp[c]
            assign[c].append((jg, bi1, bi2, l1, l2))
    return slots, assign


_SLOTS, _ASSIGN = _build_schedule()
_TOTAL_STEPS = sum(L for L, _ in _SLOTS)


def _step_width(t, L):
    """Moving-operand width (cols) at step t of an L-step sweep.

    Right-aligned: the final 3 steps of a column hit the diagonal B blocks
    (bk = 4jg+1..3) whose nonzero columns are the right 384/256/128.
    """
    rem = L - 1 - t  # steps remaining after this one
    if rem == 0:
        return 128
    if rem == 1:
        return 256
    if rem == 2:
        return 384
    return 512


_cache = {}

# Matmul dtype mode: "fp32" (4 cyc/row, exact) or "f32r" (1 cyc/row at
# N>=256, tf32-class precision ~1.5e-4 rel err).
MODE = "fp32"


def _build_nc():
    import concourse.bacc as bacc
    import concourse.mybir as mybir
    import concourse.tile as tile

    f32 = mybir.dt.float32
    mm_dt = {"fp32": mybir.dt.float32, "f32r": mybir.dt.float32r}[MODE]
    nc = bacc.Bacc()
    s_in = nc.declare_dram_parameter("S", [_TOTAL_STEPS, 128, 768], f32,
                                     isOutput=False)
    cp = nc.declare_dram_parameter("CP", [2 * len(_SLOTS), 128, 512], f32,
                                   isOutput=True)

    with tile.TileContext(nc) as tc:
        with (
            tc.tile_pool(name="st", bufs=6) as s_pool,
            tc.tile_pool(name="co", bufs=4) as c_pool,
            tc.tile_pool(name="ps", bufs=2, space="PSUM") as ps_pool,
        ):
            cursor = 0
            for s, (L, off) in enumerate(_SLOTS):
                ps1 = ps_pool.tile([128, 512], f32, tag="p1", name=f"ps1_{s}")
                ps2 = ps_pool.tile([128, 512], f32, tag="p2", name=f"ps2_{s}")
                for t in range(L):
                    w = _step_width(t, L)
                    if MODE == "f32r" and w < 256:
                        w = 256  # f32r runs at 1/4 rate below 256 cols
                    oc = 512 - w
                    st = s_pool.tile([128, 768], mm_dt, tag="s", name=f"st_{s}_{t}")
                    nc.sync.dma_start(out=st[:], in_=s_in[cursor].bitcast(mm_dt))
                    nc.tensor.matmul(
                        ps1[:, oc:], lhsT=st[:, 0:128],
                        rhs=st[:, 256 + oc:768],
                        start=(t == 0), stop=(t == L - 1),
                    )
                    if t >= off:
                        nc.tensor.matmul(
                            ps2[:, oc:], lhsT=st[:, 128:256],
                            rhs=st[:, 256 + oc:768],
                            start=(t == off), stop=(t == L - 1),
                        )
                    cursor += 1
                c1 = c_pool.tile([128, 512], f32, tag="c", name=f"c1_{s}")
                nc.vector.tensor_copy(c1[:], ps1[:])
                nc.sync.dma_start(out=cp[2 * s], in_=c1[:])
                c2 = c_pool.tile([128, 512], f32, tag="c", name=f"c2_{s}")
                nc.vector.tensor_copy(c2[:], ps2[:])
                nc.sync.dma_start(out=cp[2 * s + 1], in_=c2[:])
            assert cursor == _TOTAL_STEPS
    nc.finalize()
    return nc


def get_nc():
    key = ("nc", MODE)
    if key not in _cache:
        _cache[key] = _build_nc()
    return _cache[key]


def _pack_core(c, A4, B4):
    """Build the fused per-step stream S for core c.

    A4[bi, bk] = 128x128 block of A^T (A4[bi,bk][p,m] = A[128bi+m, 128bk+p]).
    B4[bk, jg] = 128x512 block of B.
    """
    S = np.zeros((_TOTAL_STEPS, 128, 768), dtype=np.float32)
    cursor = 0
    for s, (L, off) in enumerate(_SLOTS):
        jg, bi1, bi2, l1, l2 = _ASSIGN[c][s]
        base = 4 * jg + 4 - L  # bk at template step 0
        for t in range(L):
            bk = base + t
            row = S[cursor]
            if bk >= bi1:
                row[:, 0:128] = A4[bi1, bk]
            if t >= off and bk >= bi2:
                row[:, 128:256] = A4[bi2, bk]
            if bk >= bi1:  # steps before bi1 never feed a real matmul
                row[:, 256:768] = B4[bk, jg]
            cursor += 1
    return S


def kernel(A: np.ndarray, B: np.ndarray) -> np.ndarray:
    from concourse.bass_utils import run_bass_kernel_spmd

    A = np.asarray(A, dtype=np.float32)
    B = np.asarray(B, dtype=np.float32)

    nc = get_nc()

    # Block views for packing
    A4 = A.reshape(NB, 128, NB, 128).transpose(0, 2, 3, 1)  # [bi, bk] -> A_blk^T
    B4 = B.reshape(NB, 128, NJ, 512).transpose(0, 2, 1, 3)  # [bk, jg] -> B_blk

    in_maps = [{"S": _pack_core(c, A4, B4)} for c in range(N_CORES)]

    res = run_bass_kernel_spmd(nc, in_maps, list(range(N_CORES)))

    C = np.zeros((N, N), dtype=np.float32)
    for c in range(N_CORES):
        cpk = res.results[c]["CP"]
        for s in range(len(_SLOTS)):
            jg, bi1, bi2, l1, l2 = _ASSIGN[c][s]
            for track, bi in ((0, bi1), (1, bi2)):
                blk = cpk[2 * s + track]
                # valid columns of this super: j >= 128*bi (left of the
                # diagonal is exact zero / possibly pad garbage)
                lo = max(0, 128 * bi - 512 * jg)
                C[128 * bi:128 * (bi + 1), 512 * jg + lo:512 * (jg + 1)] = \
                    blk[:, lo:]
    return C


def _make_in_maps(A, B):
    A = np.asarray(A, dtype=np.float32)
    B = np.asarray(B, dtype=np.float32)
    A4 = A.reshape(NB, 128, NB, 128).transpose(0, 2, 3, 1)
    B4 = B.reshape(NB, 128, NJ, 512).transpose(0, 2, 1, 3)
    return [{"S": _pack_core(c, A4, B4)} for c in range(N_CORES)]
